# revision 102
# baseline (speedup 1.0000x reference)
"""AttentionBlock (GroupNorm -> 1x1 QKV -> softmax attention -> proj -> residual)
on Trainium2, data-parallel over batch: 32 images across 8 NeuronCores (4 per core).

Self-contained: hardcodes B=32, C=256, H=W=32, GROUPS=8, EPS=1e-5.
"""

import numpy as np
import ml_dtypes
import jax
from jax.experimental.shard_map import shard_map
from jax.sharding import Mesh, PartitionSpec

import concourse.bass as bass
import concourse.tile as tile
from concourse import bacc, mybir
from concourse import bass2jax

F32 = mybir.dt.float32
BF16 = mybir.dt.bfloat16
AF = mybir.ActivationFunctionType
ALU = mybir.AluOpType

NCORES = 8
B = 32
BPC = B // NCORES  # images per core
C = 256
N = 1024           # H*W
G = 8              # groups
GS = C // G        # 32 channels per group
EPS = 1e-5
P = 128
NT = C // P        # 2 channel tiles
SCALE = C ** -0.5  # 1/16

_cached = None


def _build_program(repeat=1):
    nc = bacc.Bacc("TRN2", target_bir_lowering=False, debug=False,
                   num_devices=NCORES)

    x_d = nc.dram_tensor("x", [BPC, C, N], F32, kind="ExternalInput")
    wqkT_d = nc.dram_tensor("wqkT", [P, NT, 2 * C], BF16, kind="ExternalInput")
    # wvT holds (proj_w @ v_w).T — proj is folded into V on the host, since
    # attention only mixes spatially: Wp @ (attn @ (Wv h)) = attn @ ((Wp Wv) h)
    wvT_d = nc.dram_tensor("wvT", [P, NT, C], BF16, kind="ExternalInput")
    sel_d = nc.dram_tensor("sel", [P, NT, G], F32, kind="ExternalInput")
    selb_d = nc.dram_tensor("selb", [P, C], F32, kind="ExternalInput")
    aff_d = nc.dram_tensor("aff", [P, 3 * NT], F32, kind="ExternalInput")
    ident_d = nc.dram_tensor("ident", [P, P], BF16, kind="ExternalInput")
    out_d = nc.dram_tensor("out", [BPC, C, N], F32, kind="ExternalOutput")

    with tile.TileContext(nc) as tc:
        with (
            tc.tile_pool(name="consts", bufs=1) as consts,
            tc.tile_pool(name="xp", bufs=2) as xp,
            tc.tile_pool(name="gn", bufs=2) as gn,
            tc.tile_pool(name="hp", bufs=3) as hp,
            tc.tile_pool(name="qkp", bufs=3) as qkp,
            tc.tile_pool(name="vtp", bufs=3) as vtp,
            tc.tile_pool(name="ptp", bufs=3) as ptp,
            tc.tile_pool(name="op", bufs=3) as op,
            tc.tile_pool(name="resp", bufs=2) as resp,
            tc.tile_pool(name="recp", bufs=4) as recp,
            tc.tile_pool(name="xpbp", bufs=2) as xpbp,
            tc.tile_pool(name="ps1", bufs=3, space="PSUM") as ps1,
            tc.tile_pool(name="ps2", bufs=2, space="PSUM") as ps2,
            tc.tile_pool(name="psg_p", bufs=1, space="PSUM") as psg_p,
        ):
            wqkT = consts.tile([P, NT, 2 * C], BF16)
            wvT = consts.tile([P, NT, C], BF16)
            sel = consts.tile([P, NT, G], F32)
            selb = consts.tile([P, C], F32)
            aff = consts.tile([P, 3 * NT], F32)
            ident = consts.tile([P, P], BF16)

            def emit_small_consts():
                nc.gpsimd.dma_start(sel[:], sel_d.ap())
                nc.gpsimd.dma_start(selb[:], selb_d.ap())
                nc.gpsimd.dma_start(ident[:], ident_d.ap())

            def emit_weight_loads():
                # SWDGE path runs parallel to the HWDGE x-load at startup
                nc.gpsimd.dma_start(wqkT[:], wqkT_d.ap())
                nc.gpsimd.dma_start(wvT[:], wvT_d.ap())

            def emit_x(img):
                x_sb = xp.tile([P, NT, N], F32, tag="x")
                xr = x_d.ap()[img].rearrange("(t p) n -> p t n", p=P)
                for t in range(NT):
                    for s in range(2):
                        nc.sync.dma_start(
                            x_sb[:, t, s * 512:(s + 1) * 512],
                            xr[:, t, s * 512:(s + 1) * 512])
                return x_sb

            def emit_xpb(x_sb):
                """residual base xpb = x + proj_b on the idle GPSIMD"""
                xpb_sb = xpbp.tile([P, NT, N], F32, tag="xpb")
                for t in range(NT):
                    nc.gpsimd.tensor_scalar_add(
                        xpb_sb[:, t, :], x_sb[:, t, :],
                        aff[:, 2 * NT + t:2 * NT + t + 1])
                return xpb_sb

            def emit_load(img):
                x_sb = emit_x(img)
                return x_sb, emit_xpb(x_sb)

            def emit_gn_h(x_sb, first=False):
                """GroupNorm stats -> per-channel affine -> h (bf16)."""
                bst = gn.tile([P, NT, 2, 6], F32, tag="bst")
                for t in range(NT):
                    for s in range(2):
                        nc.vector.bn_stats(
                            bst[:, t, s, :], x_sb[:, t, s * 512:(s + 1) * 512])
                cmv = gn.tile([P, NT, 2], F32, tag="cmv")
                for t in range(NT):
                    nc.vector.bn_aggr(cmv[:, t, :], bst[:, t, :, :])
                # ex2 columns: [mean_c, E[x^2]_c]
                ex2 = gn.tile([P, NT, 2], F32, tag="ex2")
                for t in range(NT):
                    nc.vector.tensor_mul(
                        ex2[:, t, 1:2], cmv[:, t, 0:1], cmv[:, t, 0:1])
                    nc.vector.tensor_add(
                        ex2[:, t, 1:2], ex2[:, t, 1:2], cmv[:, t, 1:2])
                    # image 0: Pool queue is busy with descriptor-gen at
                    # startup; DVE keeps the stats chain on the critical path
                    (nc.vector if first else nc.gpsimd).tensor_copy(
                        ex2[:, t, 0:1], cmv[:, t, 0:1])
                # group stats = (1/GS) * sel.T @ ex2 -> psum [G, 2]
                psg = psg_p.tile([G, 2], F32, tag="g")
                for t in range(NT):
                    nc.tensor.matmul(psg[:], sel[:, t, :], ex2[:, t, :],
                                     start=(t == 0), stop=(t == NT - 1))
                # gsb cols: [mean_g, rstd_g, v, tmp]; rows 8..127 zero (pad
                # for matmul).  rstd via DVE-only Newton rsqrt so Exp stays
                # the single ACT table set (no per-image table reloads).
                gsb = gn.tile([P, 4], F32, tag="gsb")
                nc.vector.memset(gsb[:], 0.0)
                nc.vector.tensor_copy(gsb[0:G, 0:1], psg[:, 0:1])
                nc.vector.tensor_mul(
                    gsb[0:G, 3:4], gsb[0:G, 0:1], gsb[0:G, 0:1])
                nc.vector.tensor_tensor(
                    gsb[0:G, 2:3], psg[:, 1:2], gsb[0:G, 3:4], ALU.subtract)
                nc.vector.tensor_scalar_add(gsb[0:G, 2:3], gsb[0:G, 2:3], EPS)
                nc.vector.reciprocal(gsb[0:G, 3:4], gsb[0:G, 2:3])
                nc.vector.tensor_scalar(
                    gsb[0:G, 1:2], gsb[0:G, 3:4], 1.0, 0.5, ALU.add, ALU.mult)
                for _ in range(2):
                    nc.vector.tensor_mul(
                        gsb[0:G, 3:4], gsb[0:G, 1:2], gsb[0:G, 1:2])
                    nc.vector.tensor_mul(
                        gsb[0:G, 3:4], gsb[0:G, 3:4], gsb[0:G, 2:3])
                    nc.vector.tensor_scalar(
                        gsb[0:G, 3:4], gsb[0:G, 3:4], -0.5, 1.5,
                        ALU.mult, ALU.add)
                    nc.vector.tensor_mul(
                        gsb[0:G, 1:2], gsb[0:G, 1:2], gsb[0:G, 3:4])
                # broadcast group -> channel: selb.T @ gsb -> [c, (mean,rstd)]
                AB = gn.tile([P, NT, 2], F32, tag="AB")
                for cu in range(NT):
                    psc = psg_p.tile([P, 2], F32, tag="g")
                    nc.tensor.matmul(psc[:], selb[:, cu * P:(cu + 1) * P],
                                     gsb[:, 0:2], start=True, stop=True)
                    # A = rstd*w ; B = b - mean*A
                    nc.vector.tensor_mul(
                        AB[:, cu, 0:1], psc[:, 1:2], aff[:, cu:cu + 1])
                    nc.vector.tensor_mul(
                        AB[:, cu, 1:2], psc[:, 0:1], AB[:, cu, 0:1])
                    nc.vector.tensor_tensor(
                        AB[:, cu, 1:2], aff[:, NT + cu:NT + cu + 1],
                        AB[:, cu, 1:2], ALU.subtract)
                # h = A*x + B   (bf16)
                h_sb = hp.tile([P, NT, N], BF16, tag="h")
                for t in range(NT):
                    nc.vector.tensor_scalar(
                        h_sb[:, t, :], x_sb[:, t, :],
                        AB[:, t, 0:1], AB[:, t, 1:2], ALU.mult, ALU.add)
                return h_sb

            def emit_qkv(h_sb):
                """q,k = Wqk @ h ; vpT = h.T @ (WpWv).T — returned as filler
                closures so the MMs can plug PE bubbles in the previous
                image's out-stage."""
                qk_sb = qkp.tile([P, 4, N], BF16, tag="qk")
                vto_sb = vtp.tile([P, G, C + 1], BF16, tag="vto")
                nc.gpsimd.memset(vto_sb[:, :, C:C + 1], 1.0)

                def qk_group(ou):
                    psq = ps2.tile([P, N], F32, tag="b")
                    for t in range(NT):
                        for nh in range(2):
                            nc.tensor.matmul(
                                psq[:, nh * 512:(nh + 1) * 512],
                                wqkT[:, t, ou * P:(ou + 1) * P],
                                h_sb[:, t, nh * 512:(nh + 1) * 512],
                                start=(t == 0), stop=(t == NT - 1))
                    # split copy across DVE and ACT to halve its latency
                    nc.vector.tensor_copy(
                        qk_sb[:, ou, 0:512], psq[:, 0:512])
                    nc.scalar.copy(qk_sb[:, ou, 512:1024], psq[:, 512:1024])

                def vt_group(j):
                    psv = psg_p.tile([P, 2, C], F32, tag="g")
                    for half in range(2):
                        nk = 2 * j + half
                        for t in range(NT):
                            nc.tensor.matmul(
                                psv[:, half, :],
                                h_sb[:, t, nk * P:(nk + 1) * P],
                                wvT[:, t, :],
                                start=(t == 0), stop=(t == NT - 1))
                    nc.vector.tensor_copy(
                        vto_sb[:, 2 * j:2 * j + 2, 0:C], psv[:])

                # all qk groups first: logits depends only on them, and
                # their DVE copies must not queue behind vT copies
                fillers = [
                    lambda: qk_group(2), lambda: qk_group(0),
                    lambda: qk_group(3), lambda: qk_group(1),
                    lambda: vt_group(0), lambda: vt_group(1),
                    lambda: vt_group(2), lambda: vt_group(3),
                ]
                return qk_sb, vto_sb, fillers

            def emit_logits(qk_sb):
                """logitsT [m, n] = k.T @ q ; PT = exp(logitsT/16)."""
                pt_sb = ptp.tile([P, 8, N], BF16, tag="pt")
                for mk in range(8):
                    psl = ps2.tile([P, N], F32, tag="b")
                    for t in range(NT):
                        for nh in range(2):
                            nc.tensor.matmul(
                                psl[:, nh * 512:(nh + 1) * 512],
                                qk_sb[:, 2 + t, mk * P:(mk + 1) * P],
                                qk_sb[:, t, nh * 512:(nh + 1) * 512],
                                start=(t == 0), stop=(t == NT - 1))
                    nc.scalar.activation(pt_sb[:, mk, :], psl[:], AF.Exp,
                                         scale=SCALE)
                return pt_sb

            def emit_out(img, vto_sb, pt_sb, xpb_sb, is_last,
                         fillers=()):
                """projT = softmax(l) @ [vpT|1]; PE-transpose + fused
                residual (res = projT.T + x + proj_b) per 128x128 block."""
                o_sb = op.tile([P, 8, C], BF16, tag="o")
                res_sb = resp.tile([P, NT, N], F32, tag="res")
                outr = out_d.ap()[img].rearrange("(t p) n -> p t n", p=P)

                def emit_tr(nk):
                    for cu in range(NT):
                        pst = ps1.tile([P, P], BF16, tag="s")
                        nc.tensor.transpose(
                            pst[:], o_sb[:, nk, cu * P:(cu + 1) * P],
                            ident[:])
                        nc.vector.tensor_tensor(
                            res_sb[:, cu, nk * P:(nk + 1) * P], pst[:],
                            xpb_sb[:, cu, nk * P:(nk + 1) * P], ALU.add)

                for nk in range(8):
                    pso = ps1.tile([P, C + 1], F32, tag="s")
                    for mk in range(8):
                        nc.tensor.matmul(
                            pso[:], pt_sb[:, mk, nk * P:(nk + 1) * P],
                            vto_sb[:, mk, :], start=(mk == 0), stop=(mk == 7))
                    rec = recp.tile([P, 1], F32, tag="rec")
                    nc.vector.reciprocal(rec[:], pso[:, C:C + 1])
                    nc.vector.tensor_scalar_mul(
                        o_sb[:, nk, :], pso[:, 0:C], rec[:])
                    # transpose blocks lag one nk so PE never waits on norms
                    if nk > 0:
                        emit_tr(nk - 1)
                    if nk < len(fillers):
                        fillers[nk]()
                    if nk == 4:
                        for cu in range(NT):
                            nc.sync.dma_start(
                                outr[:, cu, 0:512], res_sb[:, cu, 0:512])
                # ship nk4-6 before the nk7 chain resolves; only the final
                # 128-col block rides the tail-critical path
                for cu in range(NT):
                    nc.sync.dma_start(
                        outr[:, cu, 512:896], res_sb[:, cu, 512:896])
                for cu in range(NT):
                    pst = ps1.tile([P, P], BF16, tag="s")
                    nc.tensor.transpose(
                        pst[:], o_sb[:, 7, cu * P:(cu + 1) * P], ident[:])
                    nc.vector.tensor_tensor(
                        res_sb[:, cu, 7 * P:8 * P], pst[:],
                        xpb_sb[:, cu, 7 * P:8 * P], ALU.add)
                    nc.sync.dma_start(
                        outr[:, cu, 896:1024], res_sb[:, cu, 896:1024])

            # software-pipelined emission: the GroupNorm/DVE chain of image
            # i+1 is emitted before the out-stage of image i so it never
            # queues behind image i's late attention consumers.
            # warmup: trigger the Exp ACT-table DMA (~2.7us) at t=0 so it
            # is resident long before image 0's first real softmax exp
            imgs = [i % BPC for i in range(BPC * repeat)]
            # aff is consumed late (xpb is deferred past the stats chain),
            # so it rides the SWDGE queue, keeping HWDGE x-only at startup
            nc.gpsimd.dma_start(aff[:], aff_d.ap())
            emit_weight_loads()
            emit_small_consts()
            x0 = emit_x(imgs[0])
            # warmup: trigger the Exp ACT-table DMA (~2.7us) right after the
            # x-load dispatches so it is resident long before image 0's
            # first softmax exp but doesn't block the ACT DMA queue
            warm = consts.tile([P, 1], F32)
            nc.vector.memset(warm[:], 0.0)
            nc.scalar.activation(warm[:], warm[:], AF.Exp)
            h0 = emit_gn_h(x0, first=True)
            xpb0 = emit_xpb(x0)
            qk0, vto0, fs0 = emit_qkv(h0)
            for f in fs0:
                f()
            cur = (imgs[0], xpb0, qk0, vto0)
            for idx in range(len(imgs)):
                img, xpb_sb, qk_sb, vto_sb = cur
                pt_sb = emit_logits(qk_sb)
                fillers = ()
                if idx + 1 < len(imgs):
                    xn, xpbn = emit_load(imgs[idx + 1])
                    hn = emit_gn_h(xn)
                    qkn, vton, fillers = emit_qkv(hn)
                    cur = (imgs[idx + 1], xpbn, qkn, vton)
                emit_out(img, vto_sb, pt_sb, xpb_sb,
                         is_last=(idx == len(imgs) - 1), fillers=fillers)

    nc.compile()
    return nc


def _build_runner(repeat=1):
    """Build nc once and wrap it in a persistent jitted 8-core SPMD callable.

    Mirrors bass2jax.run_bass_via_pjrt, but hoists the jax.jit out of the
    per-call path so repeat invocations reuse the compiled executable.
    """
    nc = _build_program(repeat)
    bass2jax.install_neuronx_cc_hook()

    partition_name = (nc.partition_id_tensor.name
                      if nc.partition_id_tensor else None)
    in_names, out_names, out_avals = [], [], []
    for alloc in nc.m.functions[0].allocations:
        if not isinstance(alloc, mybir.MemoryLocationSet):
            continue
        name = alloc.memorylocations[0].name
        if alloc.kind == "ExternalInput":
            if name != partition_name:
                in_names.append(name)
        elif alloc.kind == "ExternalOutput":
            out_names.append(name)
            out_avals.append(jax.core.ShapedArray(
                tuple(alloc.tensor_shape), mybir.dt.np(alloc.dtype)))
    n_params = len(in_names)
    all_in_names = tuple(in_names) + tuple(out_names)
    if partition_name is not None:
        all_in_names = all_in_names + (partition_name,)

    def _body(*args):
        operands = list(args)
        if partition_name is not None:
            operands.append(bass2jax.partition_id_tensor())
        return tuple(bass2jax._bass_exec_p.bind(
            *operands,
            out_avals=tuple(out_avals),
            in_names=all_in_names,
            out_names=tuple(out_names),
            lowering_input_output_aliases=(),
            sim_require_finite=True,
            sim_require_nnan=True,
            nc=nc,
        ))

    devices = jax.devices()[:NCORES]
    mesh = Mesh(np.asarray(devices), ("core",))
    nin = n_params + len(out_names)
    sharded = jax.jit(
        shard_map(_body, mesh=mesh,
                  in_specs=(PartitionSpec("core"),) * nin,
                  out_specs=(PartitionSpec("core"),) * len(out_names),
                  check_rep=False),
        keep_unused=True,
    )
    # Not donated, so these zero "output seed" buffers are reusable across
    # calls (the kernel writes every output element).
    from jax.sharding import NamedSharding
    shard = NamedSharding(mesh, PartitionSpec("core"))
    zeros_dev = [
        jax.device_put(
            np.zeros((NCORES * a.shape[0], *a.shape[1:]), a.dtype), shard)
        for a in out_avals
    ]
    return {"sharded": sharded, "in_names": in_names,
            "out_names": out_names, "out_avals": out_avals,
            "zeros_dev": zeros_dev, "mesh": mesh, "nc": nc}


def _get_runner(repeat=1):
    global _cached
    if _cached is None:
        _cached = {}
    if repeat not in _cached:
        _cached[repeat] = _build_runner(repeat)
    return _cached[repeat]


def _run(in_maps):
    r = _get_runner()
    sharded, in_names, out_names, out_avals, zeros_dev = (
        r["sharded"], r["in_names"], r["out_names"], r["out_avals"],
        r["zeros_dev"])
    concat_in = [
        np.concatenate([np.asarray(m[name]) for m in in_maps], axis=0)
        for name in in_names
    ]
    out_arrs = sharded(*concat_in, *zeros_dev)
    return {
        name: np.asarray(out_arrs[i]).reshape(
            NCORES, *out_avals[i].shape)
        for i, name in enumerate(out_names)
    }


def _pack_c(v):
    # [C] -> [P, NT] with c = t*128 + p
    return np.ascontiguousarray(v.reshape(NT, P).T)


def _pack_w(wT):
    # [C, O] -> [P, NT, O] bf16 with c = t*128 + p
    o = wT.shape[1]
    return np.ascontiguousarray(
        wT.reshape(NT, P, o).transpose(1, 0, 2)).astype(ml_dtypes.bfloat16)


def make_in_maps(x, norm_w, norm_b, qkv_w, proj_w, proj_b):
    x = np.asarray(x, dtype=np.float32)
    norm_w = np.asarray(norm_w, dtype=np.float32)
    norm_b = np.asarray(norm_b, dtype=np.float32)
    qkv_w = np.asarray(qkv_w, dtype=np.float32)
    proj_w = np.asarray(proj_w, dtype=np.float32)
    proj_b = np.asarray(proj_b, dtype=np.float32)

    wqkT = _pack_w(qkv_w[:2 * C].T)          # [P, NT, 512]
    # fold proj into V: (Wp @ Wv).T, computed in float64 for exactness
    wvp = (proj_w.astype(np.float64) @ qkv_w[2 * C:].astype(np.float64))
    wvT = _pack_w(wvp.astype(np.float32).T)  # [P, NT, 256]

    cidx = np.arange(C)
    sel = np.zeros((P, NT, G), np.float32)
    sel[cidx % P, cidx // P, cidx // GS] = 1.0 / GS
    selb = np.zeros((P, C), np.float32)
    selb[cidx // GS, cidx] = 1.0

    aff = np.concatenate(
        [_pack_c(norm_w), _pack_c(norm_b), _pack_c(proj_b)],
        axis=1).astype(np.float32)           # [P, 6]
    ident = np.eye(P, dtype=ml_dtypes.bfloat16)

    xr = x.reshape(B, C, N)
    shared = {"wqkT": wqkT, "wvT": wvT,
              "sel": sel, "selb": selb, "aff": aff, "ident": ident}
    return [
        {"x": np.ascontiguousarray(xr[c * BPC:(c + 1) * BPC]), **shared}
        for c in range(NCORES)
    ]


def kernel(x, norm_w, norm_b, qkv_w, proj_w, proj_b):
    in_maps = make_in_maps(x, norm_w, norm_b, qkv_w, proj_w, proj_b)
    outs = _run(in_maps)
    return outs["out"].reshape(B, C, 32, 32)


# revision 108
# speedup vs baseline: 1.1110x; 1.1110x over previous
"""AttentionBlock (GroupNorm -> 1x1 QKV -> softmax attention -> proj -> residual)
on Trainium2, data-parallel over batch: 32 images across 8 NeuronCores (4 per core).

Self-contained: hardcodes B=32, C=256, H=W=32, GROUPS=8, EPS=1e-5.
"""

import numpy as np
import ml_dtypes
import jax
from jax.experimental.shard_map import shard_map
from jax.sharding import Mesh, PartitionSpec

import concourse.bass as bass
import concourse.tile as tile
from concourse import bacc, mybir
from concourse import bass2jax

F32 = mybir.dt.float32
BF16 = mybir.dt.bfloat16
AF = mybir.ActivationFunctionType
ALU = mybir.AluOpType

NCORES = 8
B = 32
BPC = B // NCORES  # images per core
C = 256
N = 1024           # H*W
G = 8              # groups
GS = C // G        # 32 channels per group
EPS = 1e-5
P = 128
NT = C // P        # 2 channel tiles
SCALE = C ** -0.5  # 1/16

_cached = None


def _build_program(repeat=1):
    nc = bacc.Bacc("TRN2", target_bir_lowering=False, debug=False,
                   num_devices=NCORES)

    x_d = nc.dram_tensor("x", [BPC, C, N], F32, kind="ExternalInput")
    wqkT_d = nc.dram_tensor("wqkT", [P, NT, 2 * C], BF16, kind="ExternalInput")
    # wvT holds (proj_w @ v_w).T — proj is folded into V on the host, since
    # attention only mixes spatially: Wp @ (attn @ (Wv h)) = attn @ ((Wp Wv) h)
    wvT_d = nc.dram_tensor("wvT", [P, NT, C], BF16, kind="ExternalInput")
    sel_d = nc.dram_tensor("sel", [P, NT, G], F32, kind="ExternalInput")
    selb_d = nc.dram_tensor("selb", [P, C], F32, kind="ExternalInput")
    aff_d = nc.dram_tensor("aff", [P, 3 * NT], F32, kind="ExternalInput")
    ident_d = nc.dram_tensor("ident", [P, P], BF16, kind="ExternalInput")
    out_d = nc.dram_tensor("out", [BPC, C, N], F32, kind="ExternalOutput")

    with tile.TileContext(nc) as tc:
        with (
            tc.tile_pool(name="consts", bufs=1) as consts,
            tc.tile_pool(name="xp", bufs=2) as xp,
            tc.tile_pool(name="gn", bufs=2) as gn,
            tc.tile_pool(name="hp", bufs=3) as hp,
            tc.tile_pool(name="qkp", bufs=3) as qkp,
            tc.tile_pool(name="vtp", bufs=3) as vtp,
            tc.tile_pool(name="ptp", bufs=3) as ptp,
            tc.tile_pool(name="op", bufs=3) as op,
            tc.tile_pool(name="resp", bufs=2) as resp,
            tc.tile_pool(name="recp", bufs=4) as recp,
            tc.tile_pool(name="xpbp", bufs=2) as xpbp,
            tc.tile_pool(name="ps1", bufs=3, space="PSUM") as ps1,
            tc.tile_pool(name="ps2", bufs=2, space="PSUM") as ps2,
            tc.tile_pool(name="psg_p", bufs=1, space="PSUM") as psg_p,
        ):
            wqkT = consts.tile([P, NT, 2 * C], BF16)
            wvT = consts.tile([P, NT, C], BF16)
            sel = consts.tile([P, NT, G], F32)
            selb = consts.tile([P, C], F32)
            aff = consts.tile([P, 3 * NT], F32)
            ident = consts.tile([P, P], BF16)

            def emit_small_consts():
                nc.gpsimd.dma_start(sel[:], sel_d.ap())
                nc.gpsimd.dma_start(selb[:], selb_d.ap())
                nc.gpsimd.dma_start(ident[:], ident_d.ap())

            def emit_weight_loads():
                # SWDGE path runs parallel to the HWDGE x-load at startup
                nc.gpsimd.dma_start(wqkT[:], wqkT_d.ap())
                nc.gpsimd.dma_start(wvT[:], wvT_d.ap())

            def emit_x(img):
                x_sb = xp.tile([P, NT, N], F32, tag="x")
                xr = x_d.ap()[img].rearrange("(t p) n -> p t n", p=P)
                for t in range(NT):
                    for s in range(2):
                        nc.sync.dma_start(
                            x_sb[:, t, s * 512:(s + 1) * 512],
                            xr[:, t, s * 512:(s + 1) * 512])
                return x_sb

            def emit_xpb(x_sb):
                """residual base xpb = x + proj_b on the idle GPSIMD"""
                xpb_sb = xpbp.tile([P, NT, N], F32, tag="xpb")
                for t in range(NT):
                    nc.gpsimd.tensor_scalar_add(
                        xpb_sb[:, t, :], x_sb[:, t, :],
                        aff[:, 2 * NT + t:2 * NT + t + 1])
                return xpb_sb

            def emit_load(img):
                x_sb = emit_x(img)
                return x_sb, emit_xpb(x_sb)

            def emit_gn_h(x_sb, first=False):
                """GroupNorm stats -> per-channel affine -> h (bf16)."""
                bst = gn.tile([P, NT, 2, 6], F32, tag="bst")
                for t in range(NT):
                    for s in range(2):
                        nc.vector.bn_stats(
                            bst[:, t, s, :], x_sb[:, t, s * 512:(s + 1) * 512])
                cmv = gn.tile([P, NT, 2], F32, tag="cmv")
                for t in range(NT):
                    nc.vector.bn_aggr(cmv[:, t, :], bst[:, t, :, :])
                # ex2 columns: [mean_c, E[x^2]_c]
                ex2 = gn.tile([P, NT, 2], F32, tag="ex2")
                for t in range(NT):
                    nc.vector.tensor_mul(
                        ex2[:, t, 1:2], cmv[:, t, 0:1], cmv[:, t, 0:1])
                    nc.vector.tensor_add(
                        ex2[:, t, 1:2], ex2[:, t, 1:2], cmv[:, t, 1:2])
                    # image 0: Pool queue is busy with descriptor-gen at
                    # startup; DVE keeps the stats chain on the critical path
                    (nc.vector if first else nc.gpsimd).tensor_copy(
                        ex2[:, t, 0:1], cmv[:, t, 0:1])
                # group stats = (1/GS) * sel.T @ ex2 -> psum [G, 2]
                psg = psg_p.tile([G, 2], F32, tag="g")
                for t in range(NT):
                    nc.tensor.matmul(psg[:], sel[:, t, :], ex2[:, t, :],
                                     start=(t == 0), stop=(t == NT - 1))
                # gsb cols: [mean_g, rstd_g, v, tmp]; rows 8..127 zero (pad
                # for matmul).  rstd via DVE-only Newton rsqrt so Exp stays
                # the single ACT table set (no per-image table reloads).
                gsb = gn.tile([P, 4], F32, tag="gsb")
                nc.vector.memset(gsb[:], 0.0)
                nc.vector.tensor_copy(gsb[0:G, 0:1], psg[:, 0:1])
                nc.vector.tensor_mul(
                    gsb[0:G, 3:4], gsb[0:G, 0:1], gsb[0:G, 0:1])
                nc.vector.tensor_tensor(
                    gsb[0:G, 2:3], psg[:, 1:2], gsb[0:G, 3:4], ALU.subtract)
                nc.vector.tensor_scalar_add(gsb[0:G, 2:3], gsb[0:G, 2:3], EPS)
                nc.vector.reciprocal(gsb[0:G, 3:4], gsb[0:G, 2:3])
                nc.vector.tensor_scalar(
                    gsb[0:G, 1:2], gsb[0:G, 3:4], 1.0, 0.5, ALU.add, ALU.mult)
                for _ in range(2):
                    nc.vector.tensor_mul(
                        gsb[0:G, 3:4], gsb[0:G, 1:2], gsb[0:G, 1:2])
                    nc.vector.tensor_mul(
                        gsb[0:G, 3:4], gsb[0:G, 3:4], gsb[0:G, 2:3])
                    nc.vector.tensor_scalar(
                        gsb[0:G, 3:4], gsb[0:G, 3:4], -0.5, 1.5,
                        ALU.mult, ALU.add)
                    nc.vector.tensor_mul(
                        gsb[0:G, 1:2], gsb[0:G, 1:2], gsb[0:G, 3:4])
                # broadcast group -> channel: selb.T @ gsb -> [c, (mean,rstd)]
                # h(t) emitted right after its own AB(t) so the first qk
                # matmuls (which only need h t=0) unblock one step earlier
                AB = gn.tile([P, NT, 2], F32, tag="AB")
                h_sb = hp.tile([P, NT, N], BF16, tag="h")
                for cu in range(NT):
                    psc = psg_p.tile([P, 2], F32, tag="g")
                    nc.tensor.matmul(psc[:], selb[:, cu * P:(cu + 1) * P],
                                     gsb[:, 0:2], start=True, stop=True)
                    # A = rstd*w ; B = b - mean*A
                    nc.vector.tensor_mul(
                        AB[:, cu, 0:1], psc[:, 1:2], aff[:, cu:cu + 1])
                    nc.vector.tensor_mul(
                        AB[:, cu, 1:2], psc[:, 0:1], AB[:, cu, 0:1])
                    nc.vector.tensor_tensor(
                        AB[:, cu, 1:2], aff[:, NT + cu:NT + cu + 1],
                        AB[:, cu, 1:2], ALU.subtract)
                    # h = A*x + B   (bf16)
                    nc.vector.tensor_scalar(
                        h_sb[:, cu, :], x_sb[:, cu, :],
                        AB[:, cu, 0:1], AB[:, cu, 1:2], ALU.mult, ALU.add)
                return h_sb

            def emit_qkv(h_sb):
                """q,k = Wqk @ h ; vpT = h.T @ (WpWv).T — returned as filler
                closures so the MMs can plug PE bubbles in the previous
                image's out-stage."""
                qk_sb = qkp.tile([P, 4, N], BF16, tag="qk")
                vto_sb = vtp.tile([P, G, C + 1], BF16, tag="vto")
                nc.gpsimd.memset(vto_sb[:, :, C:C + 1], 1.0)

                def qk_group(ou):
                    psq = ps2.tile([P, N], F32, tag="b")
                    for t in range(NT):
                        for nh in range(2):
                            nc.tensor.matmul(
                                psq[:, nh * 512:(nh + 1) * 512],
                                wqkT[:, t, ou * P:(ou + 1) * P],
                                h_sb[:, t, nh * 512:(nh + 1) * 512],
                                start=(t == 0), stop=(t == NT - 1))
                    # split copy across DVE and ACT to halve its latency
                    nc.vector.tensor_copy(
                        qk_sb[:, ou, 0:512], psq[:, 0:512])
                    nc.scalar.copy(qk_sb[:, ou, 512:1024], psq[:, 512:1024])

                def vt_group(j):
                    psv = psg_p.tile([P, 2, C], F32, tag="g")
                    for half in range(2):
                        nk = 2 * j + half
                        for t in range(NT):
                            nc.tensor.matmul(
                                psv[:, half, :],
                                h_sb[:, t, nk * P:(nk + 1) * P],
                                wvT[:, t, :],
                                start=(t == 0), stop=(t == NT - 1))
                    nc.vector.tensor_copy(
                        vto_sb[:, 2 * j:2 * j + 2, 0:C], psv[:])

                # all qk groups first: logits depends only on them, and
                # their DVE copies must not queue behind vT copies
                fillers = [
                    lambda: qk_group(2), lambda: qk_group(0),
                    lambda: qk_group(3), lambda: qk_group(1),
                    lambda: vt_group(0), lambda: vt_group(1),
                    lambda: vt_group(2), lambda: vt_group(3),
                ]
                return qk_sb, vto_sb, fillers

            def emit_logits(qk_sb):
                """logitsT [m, n] = k.T @ q ; PT = exp(logitsT/16)."""
                pt_sb = ptp.tile([P, 8, N], BF16, tag="pt")
                for mk in range(8):
                    psl = ps2.tile([P, N], F32, tag="b")
                    for t in range(NT):
                        for nh in range(2):
                            nc.tensor.matmul(
                                psl[:, nh * 512:(nh + 1) * 512],
                                qk_sb[:, 2 + t, mk * P:(mk + 1) * P],
                                qk_sb[:, t, nh * 512:(nh + 1) * 512],
                                start=(t == 0), stop=(t == NT - 1))
                    nc.scalar.activation(pt_sb[:, mk, :], psl[:], AF.Exp,
                                         scale=SCALE)
                return pt_sb

            def emit_out(img, vto_sb, pt_sb, xpb_sb, is_last,
                         fillers=()):
                """projT = softmax(l) @ [vpT|1]; PE-transpose + fused
                residual (res = projT.T + x + proj_b) per 128x128 block."""
                o_sb = op.tile([P, 8, C], BF16, tag="o")
                res_sb = resp.tile([P, NT, N], F32, tag="res")
                outr = out_d.ap()[img].rearrange("(t p) n -> p t n", p=P)

                def emit_tr(nk):
                    for cu in range(NT):
                        pst = ps1.tile([P, P], BF16, tag="s")
                        nc.tensor.transpose(
                            pst[:], o_sb[:, nk, cu * P:(cu + 1) * P],
                            ident[:])
                        nc.vector.tensor_tensor(
                            res_sb[:, cu, nk * P:(nk + 1) * P], pst[:],
                            xpb_sb[:, cu, nk * P:(nk + 1) * P], ALU.add)

                for nk in range(8):
                    pso = ps1.tile([P, C + 1], F32, tag="s")
                    for mk in range(8):
                        nc.tensor.matmul(
                            pso[:], pt_sb[:, mk, nk * P:(nk + 1) * P],
                            vto_sb[:, mk, :], start=(mk == 0), stop=(mk == 7))
                    rec = recp.tile([P, 1], F32, tag="rec")
                    nc.vector.reciprocal(rec[:], pso[:, C:C + 1])
                    nc.vector.tensor_scalar_mul(
                        o_sb[:, nk, :], pso[:, 0:C], rec[:])
                    # transpose blocks lag one nk so PE never waits on norms
                    if nk > 0:
                        emit_tr(nk - 1)
                    if nk < len(fillers):
                        fillers[nk]()
                    if nk == 4:
                        for cu in range(NT):
                            nc.sync.dma_start(
                                outr[:, cu, 0:512], res_sb[:, cu, 0:512])
                # ship nk4-6 before the nk7 chain resolves; only the final
                # 128-col block rides the tail-critical path
                for cu in range(NT):
                    nc.sync.dma_start(
                        outr[:, cu, 512:896], res_sb[:, cu, 512:896])
                for cu in range(NT):
                    pst = ps1.tile([P, P], BF16, tag="s")
                    nc.tensor.transpose(
                        pst[:], o_sb[:, 7, cu * P:(cu + 1) * P], ident[:])
                    nc.vector.tensor_tensor(
                        res_sb[:, cu, 7 * P:8 * P], pst[:],
                        xpb_sb[:, cu, 7 * P:8 * P], ALU.add)
                    nc.sync.dma_start(
                        outr[:, cu, 896:1024], res_sb[:, cu, 896:1024])

            # software-pipelined emission: the GroupNorm/DVE chain of image
            # i+1 is emitted before the out-stage of image i so it never
            # queues behind image i's late attention consumers.
            # warmup: trigger the Exp ACT-table DMA (~2.7us) at t=0 so it
            # is resident long before image 0's first real softmax exp
            imgs = [i % BPC for i in range(BPC * repeat)]
            # aff is consumed late (xpb is deferred past the stats chain),
            # so it rides the SWDGE queue, keeping HWDGE x-only at startup
            nc.gpsimd.dma_start(aff[:], aff_d.ap())
            emit_weight_loads()
            emit_small_consts()
            x0 = emit_x(imgs[0])
            # warmup: trigger the Exp ACT-table DMA (~2.7us) right after the
            # x-load dispatches so it is resident long before image 0's
            # first softmax exp but doesn't block the ACT DMA queue
            warm = consts.tile([P, 1], F32)
            nc.vector.memset(warm[:], 0.0)
            nc.scalar.activation(warm[:], warm[:], AF.Exp)
            h0 = emit_gn_h(x0, first=True)
            xpb0 = emit_xpb(x0)
            qk0, vto0, fs0 = emit_qkv(h0)
            for f in fs0:
                f()
            cur = (imgs[0], xpb0, qk0, vto0)
            for idx in range(len(imgs)):
                img, xpb_sb, qk_sb, vto_sb = cur
                pt_sb = emit_logits(qk_sb)
                fillers = ()
                if idx + 1 < len(imgs):
                    xn, xpbn = emit_load(imgs[idx + 1])
                    hn = emit_gn_h(xn)
                    qkn, vton, fillers = emit_qkv(hn)
                    cur = (imgs[idx + 1], xpbn, qkn, vton)
                emit_out(img, vto_sb, pt_sb, xpb_sb,
                         is_last=(idx == len(imgs) - 1), fillers=fillers)

    nc.compile()
    return nc


def _build_runner(repeat=1):
    """Build nc once and wrap it in a persistent jitted 8-core SPMD callable.

    Mirrors bass2jax.run_bass_via_pjrt, but hoists the jax.jit out of the
    per-call path so repeat invocations reuse the compiled executable.
    """
    nc = _build_program(repeat)
    bass2jax.install_neuronx_cc_hook()

    partition_name = (nc.partition_id_tensor.name
                      if nc.partition_id_tensor else None)
    in_names, out_names, out_avals = [], [], []
    for alloc in nc.m.functions[0].allocations:
        if not isinstance(alloc, mybir.MemoryLocationSet):
            continue
        name = alloc.memorylocations[0].name
        if alloc.kind == "ExternalInput":
            if name != partition_name:
                in_names.append(name)
        elif alloc.kind == "ExternalOutput":
            out_names.append(name)
            out_avals.append(jax.core.ShapedArray(
                tuple(alloc.tensor_shape), mybir.dt.np(alloc.dtype)))
    n_params = len(in_names)
    all_in_names = tuple(in_names) + tuple(out_names)
    if partition_name is not None:
        all_in_names = all_in_names + (partition_name,)

    def _body(*args):
        operands = list(args)
        if partition_name is not None:
            operands.append(bass2jax.partition_id_tensor())
        return tuple(bass2jax._bass_exec_p.bind(
            *operands,
            out_avals=tuple(out_avals),
            in_names=all_in_names,
            out_names=tuple(out_names),
            lowering_input_output_aliases=(),
            sim_require_finite=True,
            sim_require_nnan=True,
            nc=nc,
        ))

    devices = jax.devices()[:NCORES]
    mesh = Mesh(np.asarray(devices), ("core",))
    nin = n_params + len(out_names)
    sharded = jax.jit(
        shard_map(_body, mesh=mesh,
                  in_specs=(PartitionSpec("core"),) * nin,
                  out_specs=(PartitionSpec("core"),) * len(out_names),
                  check_rep=False),
        keep_unused=True,
    )
    # Not donated, so these zero "output seed" buffers are reusable across
    # calls (the kernel writes every output element).
    from jax.sharding import NamedSharding
    shard = NamedSharding(mesh, PartitionSpec("core"))
    zeros_dev = [
        jax.device_put(
            np.zeros((NCORES * a.shape[0], *a.shape[1:]), a.dtype), shard)
        for a in out_avals
    ]
    return {"sharded": sharded, "in_names": in_names,
            "out_names": out_names, "out_avals": out_avals,
            "zeros_dev": zeros_dev, "mesh": mesh, "nc": nc}


def _get_runner(repeat=1):
    global _cached
    if _cached is None:
        _cached = {}
    if repeat not in _cached:
        _cached[repeat] = _build_runner(repeat)
    return _cached[repeat]


def _run(in_maps):
    r = _get_runner()
    sharded, in_names, out_names, out_avals, zeros_dev = (
        r["sharded"], r["in_names"], r["out_names"], r["out_avals"],
        r["zeros_dev"])
    concat_in = [
        np.concatenate([np.asarray(m[name]) for m in in_maps], axis=0)
        for name in in_names
    ]
    out_arrs = sharded(*concat_in, *zeros_dev)
    return {
        name: np.asarray(out_arrs[i]).reshape(
            NCORES, *out_avals[i].shape)
        for i, name in enumerate(out_names)
    }


def _pack_c(v):
    # [C] -> [P, NT] with c = t*128 + p
    return np.ascontiguousarray(v.reshape(NT, P).T)


def _pack_w(wT):
    # [C, O] -> [P, NT, O] bf16 with c = t*128 + p
    o = wT.shape[1]
    return np.ascontiguousarray(
        wT.reshape(NT, P, o).transpose(1, 0, 2)).astype(ml_dtypes.bfloat16)


def make_in_maps(x, norm_w, norm_b, qkv_w, proj_w, proj_b):
    x = np.asarray(x, dtype=np.float32)
    norm_w = np.asarray(norm_w, dtype=np.float32)
    norm_b = np.asarray(norm_b, dtype=np.float32)
    qkv_w = np.asarray(qkv_w, dtype=np.float32)
    proj_w = np.asarray(proj_w, dtype=np.float32)
    proj_b = np.asarray(proj_b, dtype=np.float32)

    wqkT = _pack_w(qkv_w[:2 * C].T)          # [P, NT, 512]
    # fold proj into V: (Wp @ Wv).T, computed in float64 for exactness
    wvp = (proj_w.astype(np.float64) @ qkv_w[2 * C:].astype(np.float64))
    wvT = _pack_w(wvp.astype(np.float32).T)  # [P, NT, 256]

    cidx = np.arange(C)
    sel = np.zeros((P, NT, G), np.float32)
    sel[cidx % P, cidx // P, cidx // GS] = 1.0 / GS
    selb = np.zeros((P, C), np.float32)
    selb[cidx // GS, cidx] = 1.0

    aff = np.concatenate(
        [_pack_c(norm_w), _pack_c(norm_b), _pack_c(proj_b)],
        axis=1).astype(np.float32)           # [P, 6]
    ident = np.eye(P, dtype=ml_dtypes.bfloat16)

    xr = x.reshape(B, C, N)
    shared = {"wqkT": wqkT, "wvT": wvT,
              "sel": sel, "selb": selb, "aff": aff, "ident": ident}
    return [
        {"x": np.ascontiguousarray(xr[c * BPC:(c + 1) * BPC]), **shared}
        for c in range(NCORES)
    ]


def kernel(x, norm_w, norm_b, qkv_w, proj_w, proj_b):
    in_maps = make_in_maps(x, norm_w, norm_b, qkv_w, proj_w, proj_b)
    outs = _run(in_maps)
    return outs["out"].reshape(B, C, 32, 32)
